# revision 12
# baseline (speedup 1.0000x reference)
"""Bass/Trainium2 kernel for BiasedMultiheadAttention.

Full shapes: x [2, 2048, 1024], attn_bias [2, 16, 2048, 2048],
in_proj_weight [3072, 1024], out_w [1024, 1024].

Sharding over 8 cores: core c handles batch b = c // 4 and the 4 heads
h0 = 4*(c%4) .. h0+3 (data parallel on B, tensor parallel on H).  Each
core computes its Q/K/V projection slice, full attention for its heads,
and a partial output projection over its 256 d-dims; the host sums the
4 partials per batch and adds out_b.

Device-side layout notes:
 - Scores are computed transposed (S^T [k, q]) so the softmax key dim is
   on partitions; the additive bias becomes multiplicative:
   P = exp(S^T) * expb^T, with expb = exp(attn_bias) pre-computed on the
   host (masked key rows zeroed — this also applies key_padding_mask).
 - Key tiles that are fully masked (the trailing 128 positions with the
   default mask) are skipped entirely: no S matmul, no exp, no bias DMA,
   no PV matmul, no V projection for those rows.
 - A ones-column is appended to V so the PV matmul also produces the
   softmax denominator Z in PSUM partition 64.
 - 1/sqrt(head_dim) is folded into Wq/bq on the host; Q/K biases are
   folded into the projection matmul as a rank-1 ones-row term.
 - expb is pre-packed on the host as one contiguous 512KB block per
   (head-pair, q-block, key-tile); the bias stream alternates between
   the sync and gpsimd DMA queues (one queue cannot sustain the rate).
 - Work interleaving: only Q/K for head-pair 0 runs before attention
   starts; the V projection tiles and the remaining Q/K pairs are
   emitted inside the first attention section (the tensor engine has
   slack there while the scalar engine runs exp), and the output
   projection for q-block 0 overlaps the last attention section.
 - Engine balance: exp is ScalarE-only (the critical resource); the
   exp*bias multiply, PSUM evacuations and normalize run on DVE; the
   1/Z broadcast and DMA issue run on GPSIMD.
 - All matmul operands are fp16 (accumulation in fp32 PSUM).
"""

import numpy as np
from contextlib import ExitStack

P = 128
HD = 64

# full-problem config (hardcoded per contract)
FULL_B = 2
FULL_L = 2048
FULL_D = 1024
FULL_H = 16
N_CORES = 8
CPG = N_CORES // FULL_B          # cores per batch group
FULL_NH = FULL_H // CPG          # heads per core
SCALE = 1.0 / np.sqrt(HD)


def build_nc(L=FULL_L, D=FULL_D, NH=FULL_NH, LTK=15):
    """Build the per-core bass program (SPMD: same program on all cores).

    LTK = number of 128-wide key tiles that contain any unmasked key.
    """
    import concourse.tile as tile
    from concourse import bacc, mybir

    F16, F32 = mybir.dt.float16, mybir.dt.float32
    Act = mybir.ActivationFunctionType

    LT = L // P            # token tiles (queries / output rows)
    DKT = D // P           # input-dim contraction tiles
    NPAIR = NH // 2        # head pairs
    QKM = 2 * NPAIR        # 128-wide feature tiles for Q then K
    QB = min(1024, L)      # q block width in phase 2
    NQB = L // QB
    NB5 = L // 512         # 512-wide token blocks (projections)
    EN = D // 512          # 512-wide output-feature blocks (out proj)
    VW = NH * HD           # v feature width

    nc = bacc.Bacc("TRN2", target_bir_lowering=False, debug=False)
    xT = nc.dram_tensor("xT", [D + 1, L], F16, kind="ExternalInput").ap()
    wqk = nc.dram_tensor("wqk", [D, QKM * P], F16, kind="ExternalInput").ap()
    bqk = nc.dram_tensor("bqk", [1, QKM * P], F16, kind="ExternalInput").ap()
    wv = nc.dram_tensor("wv", [D + 1, VW], F16, kind="ExternalInput").ap()
    wo = nc.dram_tensor("wo", [NH * HD, D], F16, kind="ExternalInput").ap()
    # packed bias: tile t = (hp*NQB + qb)*LTK + kt holds, per partition p,
    # [h2=0 row | h2=1 row] of the transposed exp-bias (see prepare_in_maps)
    expb = nc.dram_tensor("expb", [NPAIR * NQB * LTK, P, 2 * QB], F16,
                          kind="ExternalInput").ap()
    outp = nc.dram_tensor("outp", [L, D], F16, kind="ExternalOutput").ap()

    with tile.TileContext(nc) as tc, ExitStack() as ctx:
        const = ctx.enter_context(tc.tile_pool(name="const", bufs=1))

        # --- persistent SBUF tensors ---
        xT_sb = [const.tile([P, L], F16, tag=f"xt{i}", name=f"xt{i}") for i in range(DKT)]
        xT_ones = const.tile([1, L], F16, tag="xt_ones")
        wqk_sb = [const.tile([P, QKM * P], F16, tag=f"wqk{i}", name=f"wqk{i}") for i in range(DKT)]
        bqk_sb = const.tile([1, QKM * P], F16, tag="bqk")
        wv_sb = [const.tile([P, VW], F16, tag=f"wv{i}", name=f"wv{i}") for i in range(DKT)]
        wv_ones = const.tile([1, VW], F16, tag="wv_ones")
        wo_sb = [const.tile([P, D], F16, tag=f"wo{hp}", name=f"wo{hp}") for hp in range(NPAIR)]
        qkT_sb = [const.tile([P, L], F16, tag=f"qk{m}", name=f"qk{m}") for m in range(QKM)]
        # per-head V~ = [V_h (64) | ones (64)]: the ones columns make the PV
        # matmul emit the softmax denominator Z replicated on PSUM partition
        # 64 (Z row is DMA-shifted to partition 0 for the reciprocal)
        v_sb = [const.tile([P, NH, P], F16, tag=f"v{t}", name=f"v{t}") for t in range(LTK)]
        # attnT packed per head pair: even head on partitions 0-63 (written
        # directly by the lane-aligned normalize multiply), odd head on
        # 64-127 (placed by a partition-shifting DMA from staging)
        attnT_sb = [const.tile([P, L], F16, tag=f"at{hp}", name=f"at{hp}") for hp in range(NPAIR)]
        stg_sb = [const.tile([P, L], F16, tag=f"stg{hp}", name=f"stg{hp}") for hp in range(NPAIR)]

        # initial loads split across the two issue engines
        for i in range(DKT):
            nc.sync.dma_start(out=xT_sb[i][:, :], in_=xT[i * P:(i + 1) * P, :])
            nc.gpsimd.dma_start(out=wqk_sb[i][:, :], in_=wqk[i * P:(i + 1) * P, :])
            nc.gpsimd.dma_start(out=wv_sb[i][:, :], in_=wv[i * P:(i + 1) * P, :])
        nc.gpsimd.dma_start(out=xT_ones[:, :], in_=xT[D:D + 1, :])
        nc.gpsimd.dma_start(out=wv_ones[:, :], in_=wv[D:D + 1, :])
        nc.gpsimd.dma_start(out=bqk_sb[:, :], in_=bqk)
        for hp in range(NPAIR):
            nc.gpsimd.dma_start(out=wo_sb[hp][:, :], in_=wo[2 * hp * HD:(2 * hp + 2) * HD, :])

        # --- one PSUM pool for all phases (slots keyed by tag) so the
        # scheduler can overlap phase boundaries (pool scopes serialize) ---
        ps_all = ctx.enter_context(tc.tile_pool(name="psum", bufs=2, space="PSUM"))

        def qk_block(m, nb, early):
            """One 128-feature x 512-token block of the Q/K projection.

            Uses the "s" PSUM tag: its slots are recycled quickly by exp, so
            deferred blocks emitted inside the attention loop don't chain
            behind the long-lived attention accumulators ("apv" tag)."""
            acc = ps_all.tile([P, 512], F32, tag="s", bufs=2, name="acc")
            for kt in range(DKT):
                nc.tensor.matmul(
                    acc[:, :],
                    lhsT=wqk_sb[kt][:, m * P:(m + 1) * P],
                    rhs=xT_sb[kt][:, nb * 512:(nb + 1) * 512],
                    start=(kt == 0),
                    stop=False,
                )
            # rank-1 bias add: ones row of x times the bias row
            nc.tensor.matmul(
                acc[:, :],
                lhsT=bqk_sb[:, m * P:(m + 1) * P],
                rhs=xT_ones[:, nb * 512:(nb + 1) * 512],
                start=False,
                stop=True,
            )
            dst = qkT_sb[m][:, nb * 512:(nb + 1) * 512]
            if early:
                # pre-attention: ScalarE is idle, use it
                nc.scalar.copy(dst, acc[:, :])
            else:
                # mid-attention: keep ScalarE free for exp
                nc.vector.tensor_copy(dst, acc[:, :])

        def v_block(t):
            """V projection for token tile t, layout [tok, head, 64+ones]."""
            acc = ps_all.tile([P, VW], F32, tag="s", bufs=2, name="acc")
            for kt in range(DKT):
                nc.tensor.matmul(
                    acc[:, :],
                    lhsT=xT_sb[kt][:, t * P:(t + 1) * P],
                    rhs=wv_sb[kt][:, :],
                    start=(kt == 0),
                    stop=False,
                )
            nc.tensor.matmul(
                acc[:, :],
                lhsT=xT_ones[:, t * P:(t + 1) * P],
                rhs=wv_ones[:, :],
                start=False,
                stop=True,
            )
            nc.vector.memset(v_sb[t][:, :, HD:HD + 1], 1.0)
            nc.vector.tensor_copy(
                v_sb[t][:, :, 0:HD],
                acc[:, :].rearrange("p (h d) -> p h d", h=NH),
            )

        # --- prologue: just enough projection work to start attention ---
        for nb in range(NB5):
            qk_block(0, nb, early=True)
        for nb in range(NB5):
            qk_block(NPAIR, nb, early=True)
        v_block(0)

        # deferred projection work, interleaved into the first attention
        # section where the tensor engine has slack
        deferred = [("v", t) for t in range(1, LTK)]
        deferred += [("qk", m, nb) for m in [m for hp in range(1, NPAIR)
                                             for m in (hp, NPAIR + hp)]
                     for nb in range(NB5)]

        with tc.tile_pool(name="ebp", bufs=6) as ebp, \
             tc.tile_pool(name="ep", bufs=4) as epool, \
             tc.tile_pool(name="pp", bufs=4) as ppool, \
             tc.tile_pool(name="zp", bufs=2) as zpool, \
             tc.tile_pool(name="avp", bufs=3) as avpool, \
             tc.tile_pool(name="zrp", bufs=4) as zrpool, \
             tc.tile_pool(name="op", bufs=3) as opool:

            def ph3_block(t):
                """Output projection for token tile t (all heads)."""
                ot = opool.tile([P, D], F16, tag="ot")
                for en in range(EN):
                    acc = ps_all.tile([P, 512], F32, tag="apv", bufs=2, name="oacc")
                    for hp2 in range(NPAIR):
                        nc.tensor.matmul(
                            acc[:, :],
                            lhsT=attnT_sb[hp2][:, t * P:(t + 1) * P],
                            rhs=wo_sb[hp2][:, en * 512:(en + 1) * 512],
                            start=(hp2 == 0),
                            stop=(hp2 == NPAIR - 1),
                        )
                    # split PSUM evacuation between Scalar and Vector
                    if en == 0:
                        nc.scalar.copy(ot[:, en * 512:(en + 1) * 512], acc[:, :])
                    else:
                        nc.vector.tensor_copy(ot[:, en * 512:(en + 1) * 512], acc[:, :])
                nc.gpsimd.dma_start(
                    out=outp[t * P:(t + 1) * P, :], in_=ot[:, :])

            first = True
            for qb in range(NQB):
                for hp in range(NPAIR):
                    apv = [ps_all.tile([HD + 1, QB], F32, tag="apv", bufs=2, name="apv")
                           for _ in range(2)]
                    for kt in range(LTK):
                        bt = ebp.tile([P, 2 * QB], F16, tag="eb", name="Bt")
                        dma_eng = nc.sync if kt % 2 == 0 else nc.gpsimd
                        dma_eng.dma_start(
                            out=bt[:, :],
                            in_=expb[(hp * NQB + qb) * LTK + kt],
                        )
                        # S matmuls for both heads interleaved j-major so the
                        # (0,0)/(64,0) row-group pairs sit adjacent in the PE
                        # queue and execute concurrently (halved array, 2x).
                        Ss = [ps_all.tile([P, QB], F32, tag="s", bufs=2, name="S")
                              for _ in range(2)]
                        for j in range(QB // 512):
                            for h2 in range(2):
                                ps = slice(HD * h2, HD * (h2 + 1))
                                nc.tensor.matmul(
                                    Ss[h2][:, j * 512:(j + 1) * 512],
                                    lhsT=qkT_sb[NPAIR + hp][ps, kt * P:(kt + 1) * P],
                                    rhs=qkT_sb[hp][ps, qb * QB + j * 512:qb * QB + (j + 1) * 512],
                                    start=True,
                                    stop=True,
                                )
                        Pts = []
                        for h2 in range(2):
                            E = epool.tile([P, QB], F16, tag="e", name="E")
                            nc.scalar.activation(E[:, :], Ss[h2][:, :], Act.Exp)
                            Pt = ppool.tile([P, QB], F16, tag="p", name="Pt")
                            nc.vector.tensor_mul(
                                Pt[:, :], E[:, :], bt[:, h2 * QB:(h2 + 1) * QB])
                            Pts.append(Pt)
                        for h2 in range(2):
                            h = 2 * hp + h2
                            for j in range(QB // 512):
                                nc.tensor.matmul(
                                    apv[h2][:, j * 512:(j + 1) * 512],
                                    lhsT=v_sb[kt][:, h, 0:HD + 1],
                                    rhs=Pts[h2][:, j * 512:(j + 1) * 512],
                                    start=(kt == 0),
                                    stop=(kt == LTK - 1),
                                )
                        if first:
                            # trail deferred projection work behind this
                            # iteration: V for the next key tile first, then
                            # the remaining Q/K feature tiles
                            if kt + 1 < LTK and deferred:
                                kind = deferred.pop(0)
                                v_block(kind[1]) if kind[0] == "v" else qk_block(kind[1], kind[2], False)
                            # late in the section there is more PE slack
                            if kt >= LTK - 6 and deferred:
                                kind = deferred.pop(0)
                                v_block(kind[1]) if kind[0] == "v" else qk_block(kind[1], kind[2], False)
                    while deferred:
                        kind = deferred.pop(0)
                        v_block(kind[1]) if kind[0] == "v" else qk_block(kind[1], kind[2], False)
                    first = False
                    # drain: fast-release copies free the PSUM accumulators;
                    # Z rows (PSUM partition 64) are DMA-shifted to SBUF
                    # partitions 0/1, one batched reciprocal serves both
                    # heads, GPSIMD broadcasts 1/Z, DVE normalizes.
                    avs = []
                    zv = zpool.tile([2, QB], F32, tag="z")
                    for h2 in range(2):
                        av = avpool.tile([HD + 1, QB], F32, tag="av")
                        nc.vector.tensor_copy(av[:, :], apv[h2][:, :])
                        nc.gpsimd.dma_start(out=zv[h2:h2 + 1, :], in_=av[HD:HD + 1, :])
                        avs.append(av)
                    zi = zpool.tile([2, QB], F32, tag="zi")
                    nc.vector.reciprocal_approx_fast(out=zi[:, :], in_=zv[:, :])
                    # partition_broadcast input must sit at partition 0:
                    # shuffle head 1's row down with a small SBUF DMA
                    zi1 = zpool.tile([1, QB], F32, tag="zi1")
                    nc.gpsimd.dma_start(out=zi1[:, :], in_=zi[1:2, :])
                    for h2 in range(2):
                        zrep = zrpool.tile([HD, QB], F32, tag="zr")
                        nc.gpsimd.partition_broadcast(
                            zrep[:, :], zi[0:1, :] if h2 == 0 else zi1[:, :])
                        dst = stg_sb[hp] if h2 == 1 else attnT_sb[hp]
                        nc.vector.tensor_mul(
                            dst[0:HD, qb * QB:(qb + 1) * QB],
                            avs[h2][0:HD, :],
                            zrep[:, :],
                        )
                        if h2 == 1:
                            nc.gpsimd.dma_start(
                                out=attnT_sb[hp][HD:P, qb * QB:(qb + 1) * QB],
                                in_=stg_sb[hp][0:HD, qb * QB:(qb + 1) * QB],
                            )
                # after the first head-pair section of the LAST q block, the
                # previous q block's attnT is complete for all head pairs —
                # its output projection overlaps the remaining attention.
                if qb > 0:
                    for t in range((qb - 1) * QB // P, qb * QB // P):
                        ph3_block(t)
            for t in range((NQB - 1) * QB // P, NQB * QB // P):
                ph3_block(t)

    nc.compile()
    return nc


def _detect_ltk(key_padding_mask):
    """Number of 128-wide key tiles containing any unmasked key (max over
    batch rows so the SPMD program is shared)."""
    m = np.asarray(key_padding_mask)
    B, L = m.shape
    lt = L // P
    ltk = 0
    for t in range(lt):
        if not m[:, t * P:(t + 1) * P].all():
            ltk = t + 1
    return max(ltk, 1)


def prepare_in_maps(x, key_padding_mask, attn_bias, in_proj_weight, in_proj_bias,
                    out_w, n_cores=N_CORES):
    """Host-side sharding / layout prep. Returns (list of per-core input
    dicts, LTK)."""
    x = np.asarray(x, dtype=np.float32)
    key_padding_mask = np.asarray(key_padding_mask)
    in_proj_weight = np.asarray(in_proj_weight, dtype=np.float32)
    in_proj_bias = np.asarray(in_proj_bias, dtype=np.float32)
    out_w = np.asarray(out_w, dtype=np.float32)

    B, L, D = x.shape
    H = np.asarray(attn_bias).shape[1] if hasattr(attn_bias, "shape") else FULL_H
    cpg = n_cores // B
    NH = H // cpg
    NPAIR = NH // 2
    QKM = 2 * NPAIR
    QB = min(1024, L)
    NQB = L // QB
    LTK = _detect_ltk(key_padding_mask)

    xT_by_b = []
    for b in range(B):
        xt = np.empty((D + 1, L), np.float16)
        xt[:D] = x[b].T
        xt[D] = 1.0
        xT_by_b.append(xt)

    woT = out_w.T  # [d, e]

    in_maps = []
    for c in range(n_cores):
        b = c // cpg
        h0 = (c % cpg) * NH
        fs = slice(h0 * HD, (h0 + NH) * HD)
        wq = in_proj_weight[0:D][fs] * SCALE
        wk = in_proj_weight[D:2 * D][fs]
        wvm = in_proj_weight[2 * D:3 * D][fs]
        bq = in_proj_bias[0:D][fs] * SCALE
        bk = in_proj_bias[D:2 * D][fs]
        bv = in_proj_bias[2 * D:3 * D][fs]

        wqkh = np.ascontiguousarray(
            np.concatenate([wq, wk], axis=0).T, dtype=np.float16)   # [D, QKM*P]
        bqkh = np.concatenate([bq, bk]).reshape(1, QKM * P).astype(np.float16)
        wvh = np.empty((D + 1, NH * HD), np.float16)
        wvh[:D] = wvm.T
        wvh[D] = bv
        woh = np.ascontiguousarray(woT[fs], dtype=np.float16)       # [NH*HD, D]

        # packed transposed exp-bias: tile (hp, qb, kt) -> [P, 2*QB] where
        # partition p holds [expb^T_{2hp}[kt*P+p, qb*QB:...] |
        #                    expb^T_{2hp+1}[kt*P+p, ...]]
        mask_b = key_padding_mask[b]
        eb = np.empty((NPAIR, NQB, LTK, P, 2, QB), np.float16)
        for hp in range(NPAIR):
            for h2 in range(2):
                e32 = np.exp(np.asarray(attn_bias[b, h0 + 2 * hp + h2],
                                        dtype=np.float32))
                ebt = np.ascontiguousarray(e32.T, dtype=np.float16)  # [k, q]
                ebt[mask_b] = 0.0
                # [k, q] -> [kt, P, qb, QB] -> assign
                v = ebt[:LTK * P].reshape(LTK, P, NQB, QB)
                eb[hp, :, :, :, h2, :] = v.transpose(2, 0, 1, 3)
        eb = np.ascontiguousarray(eb.reshape(NPAIR * NQB * LTK, P, 2 * QB))

        in_maps.append({
            "xT": xT_by_b[b],
            "wqk": wqkh,
            "bqk": bqkh,
            "wv": wvh,
            "wo": woh,
            "expb": eb,
        })
    return in_maps, LTK


_NC_CACHE = {}


def _get_nc(ltk=15):
    key = (FULL_L, FULL_D, FULL_NH, ltk)
    if key not in _NC_CACHE:
        _NC_CACHE[key] = build_nc(FULL_L, FULL_D, FULL_NH, ltk)
    return _NC_CACHE[key]


def gather_output(results, out_b, B=FULL_B, n_cores=N_CORES):
    cpg = n_cores // B
    out = None
    for c in range(n_cores):
        o = results[c]["outp"]
        if out is None:
            L, D = o.shape
            out = np.zeros((B, L, D), np.float32)
        out[c // cpg] += np.asarray(o, dtype=np.float32)
    out += np.asarray(out_b, dtype=np.float32)
    return out


def kernel(x, key_padding_mask, attn_bias, in_proj_weight, in_proj_bias,
           out_w, out_b):
    from concourse import bass_utils

    in_maps, ltk = prepare_in_maps(x, key_padding_mask, attn_bias,
                                   in_proj_weight, in_proj_bias, out_w)
    nc = _get_nc(ltk)
    res = bass_utils.run_bass_kernel_spmd(
        nc, in_maps, core_ids=list(range(N_CORES)), trace=False)
    return gather_output(res.results, out_b)
